# revision 32
# baseline (speedup 1.0000x reference)
"""Trainium2 Bass kernel for single-head attention (B=4, S=4096, C=D=512).

Sharding: 8 cores = 4 batches x 2 query-halves. Each core receives x in
BOTH orientations (host-pre-transposed xT tiles for contractions over
channels, and row-major xn tiles for the P@x contraction over keys),
ROLLED so its query half occupies rows 0..2047 (attention over keys is
order-invariant, so rolling keys is exact).

Both weight pairs fold on the host, and BOTH remaining projections are
reassociated to the query side (exact algebra, not an approximation):

  score_qk = (x_q Wq + bq)(x_k Wk + bk)^T
           = [x_q (Wq Wk^T) + bq Wk^T] x_k^T + const(q)   [cancels in softmax]
  out      = (P/l)(x Wv + bv) Wo + bo
           = ((P x)/l) (Wv Wo) + (bv Wo + bo)              [reassociated]

With M = Wq Wk^T and N = Wv Wo precomputed f32 on the host:
  - NO K projection: score matmuls contract q~ = x M + bq Wk^T directly
    against the resident xT chunks,
  - NO V projection over the 4096 keys (which was the one piece of
    work duplicated across a core pair): the attention accumulation
    computes at'[c, q] = sum_k x[k, c] P[q, k] with lhsT = xn key-tile
    slices, and N is applied afterwards per 512-query block
    (po[q, d] = sum_c at'[c, q] N[c, d]) -- query-side work only,
  - bq enters via the q~ copy's bias port, bk cancels in softmax, bv+bo
    are added on the host after gather.

Per-core PE work: ~247us of matmul streaming (scores 109 + P@x 109 +
q~ 13.7 + N-apply 13.7 + l-fold ~1).

On-chip layout notes:
  - xT is host-tiled ([rg, dc, p, s], dense 128KB chunks); xn row-groups
    DMA via the "(j p) c -> p j c" rearrange; both are the persistent
    matmul operands (no on-chip transposes or copies).
  - Scores are computed transposed (scoreT[s, q]) so exp(scoreT) feeds
    the P@x matmuls directly (as rhs) with no per-tile transposes.
  - Row sums l[q] accumulate on the DVE; one tiny bf16 matmul per
    128-query block folds the partition sum AND the transpose of l;
    1/l is applied via a per-partition scale AP.
  - The s-loop is software-pipelined: score matmuls for key-tile st+1/st+2
    are issued before the exp(st)-consuming matmuls so the in-order PE
    never waits on the ScalarE.
  - bf16 output DMA (adds <= 2^-8 relative rounding; metric stays ~1e-2
    << the 2e-2 gate) halves output traffic.
"""

import sys

for _p in ("/opt/trn_rl_repo", "/root/.axon_site/_ro/trn_rl_repo"):
    if _p not in sys.path:
        sys.path.append(_p)

import numpy as np
import ml_dtypes
import concourse.bacc as bacc
import concourse.mybir as mybir
import concourse.tile as tile
from concourse.bass_utils import run_bass_kernel_spmd

F32 = mybir.dt.float32
BF16 = mybir.dt.bfloat16

MM_DT = BF16

B, S, C, D = 4, 4096, 512, 512
Q = S // 2          # queries per core
N_CORES = 8
SCALE = float(D) ** -0.5
QB = 512            # query block (psum bank width in fp32)
N_QB = Q // QB      # 4 query blocks per core
N_ST = S // 128     # 32 key tiles
N_DC = C // 128     # 4 contraction chunks
N_RG = S // 512     # 8 row groups


def _build_program():
    nc = bacc.Bacc(None, target_bir_lowering=False, debug=False)

    # host-transposed AND host-tiled: x[rg, dc, p, s] = xT[dc*128+p, rg*512+s]
    x = nc.dram_tensor("x", [N_RG, N_DC, 128, 512], BF16, kind="ExternalInput")
    xn_dram = nc.dram_tensor("xn", [S, C], BF16, kind="ExternalInput")  # row-major
    w_dram = {
        name: nc.dram_tensor(name, [C, D], BF16, kind="ExternalInput")
        for name in ("M", "N")
    }
    bq_dram = nc.dram_tensor("bq", [D], F32, kind="ExternalInput")  # bq Wk^T
    out = nc.dram_tensor("out", [Q, D], BF16, kind="ExternalOutput")

    ActFn = mybir.ActivationFunctionType

    with tile.TileContext(nc) as tc:
        persist = tc.alloc_tile_pool(name="persist", bufs=1)
        const = tc.alloc_tile_pool(name="const", bufs=1)

        ones_f32 = const.tile([128, 128], F32, tag="ones_f32")
        nc.vector.memset(ones_f32[:], 1.0)
        ones_bf = const.tile([128, 1], MM_DT, tag="ones_bf")
        nc.vector.memset(ones_bf[:], 1.0)
        bqT = const.tile([128, N_DC], F32, tag="bqT")
        # tiny 4B-element bias DMAs, needed by the very first q~ copy
        for g in range(N_DC):
            nc.gpsimd.dma_start(bqT[:, g : g + 1],
                                bq_dram[g * 128 : (g + 1) * 128].unsqueeze(1))

        wts = {}

        def emit_weight(name, engine):
            wt = persist.tile([128, N_DC, D], MM_DT, tag=f"w_{name}", name=f"w_{name}")
            for dc in range(N_DC):
                engine.dma_start(wt[:, dc, :], w_dram[name][dc * 128 : (dc + 1) * 128, :])
            wts[name] = wt

        # ---- persistent activations ----
        xT = persist.tile([128, N_DC, S], BF16, tag="xT")   # xT[p, dc, s] = x[s, dc*128+p]
        xn = persist.tile([128, N_ST, C], BF16, tag="xn")   # xn[p, i, c] = x[i*128+p, c]

        def emit_xT(rg, eng):
            for dc in range(N_DC):
                eng.dma_start(xT[:, dc, rg * 512 : (rg + 1) * 512], x[rg, dc])

        def emit_xn(rg, eng):
            eng.dma_start(
                xn[:, rg * 4 : (rg + 1) * 4, :],
                xn_dram[rg * 512 : (rg + 1) * 512, :].rearrange("(j p) c -> p j c", j=4))

        # DMA order is deadline-driven: the q~ projection (M + xT rg0) gates
        # everything; then scores consume xT rgs and P@x consumes xn rgs in
        # lockstep during query-block 0.  N is needed only ~60us in.
        emit_xT(0, nc.sync)
        emit_weight("M", nc.scalar)
        for j in range(2):              # first two key tiles, fine-grained
            nc.sync.dma_start(xn[:, j, :], xn_dram[j * 128 : (j + 1) * 128, :])
        for j in range(2, 4):
            nc.scalar.dma_start(xn[:, j, :], xn_dram[j * 128 : (j + 1) * 128, :])

        warm = const.tile([1, 1], F32, tag="warm")
        nc.scalar.activation(warm[:], ones_f32[0:1, 0:1], ActFn.Exp, scale=1.0)

        emit_xT(1, nc.sync)
        emit_xn(1, nc.scalar)
        emit_xn(2, nc.sync)
        emit_xT(2, nc.scalar)
        emit_xT(3, nc.sync)
        emit_xn(3, nc.scalar)
        emit_xn(4, nc.sync)
        emit_xT(4, nc.scalar)
        emit_xT(5, nc.sync)
        emit_xn(5, nc.scalar)
        emit_xn(6, nc.sync)
        emit_xT(6, nc.scalar)
        emit_xT(7, nc.sync)
        emit_xn(7, nc.scalar)
        emit_weight("N", nc.scalar)

        # ================= attention =================
        with tc.tile_pool(name="qT", bufs=2) as qTp, \
             tc.tile_pool(name="pT", bufs=8) as pTp, \
             tc.tile_pool(name="rl", bufs=2) as rlp, \
             tc.tile_pool(name="atb", bufs=2) as atbp, \
             tc.tile_pool(name="osb", bufs=4) as osbp, \
             tc.tile_pool(name="ps_at", bufs=4, space="PSUM") as ps_atp, \
             tc.tile_pool(name="ps_s", bufs=3, space="PSUM") as ps_sp, \
             tc.tile_pool(name="ps_l", bufs=1, space="PSUM") as ps_lp:

            def emit_qproj(qb):
                # q~ = x M + bq Wk^T for one 512-query block, straight from
                # the resident xT columns [qb*512, (qb+1)*512).
                qT = qTp.tile([128, N_DC, 512], MM_DT, tag="qT", name=f"qT{qb}")
                for g in range(N_DC):
                    pq = ps_sp.tile([128, 512], F32, tag="ss", name=f"pq{qb}_{g}")
                    for dc in range(N_DC):
                        nc.tensor.matmul(pq[:], wts["M"][:, dc, g * 128 : (g + 1) * 128],
                                         xT[:, dc, qb * 512 : (qb + 1) * 512],
                                         start=(dc == 0), stop=(dc == N_DC - 1))
                    nc.scalar.activation(qT[:, g, :], pq[:], ActFn.Identity,
                                         bias=bqT[:, g : g + 1])
                return qT

            def emit_score(qb, st, qT):
                # scoreT[s in st, q] = sum_dc xT[:, dc, st]^T qT[:, dc, :]
                ss = ps_sp.tile([128, 512], F32, tag="ss", name=f"ss{qb}_{st}")
                for dc in range(N_DC):
                    nc.tensor.matmul(ss[:], xT[:, dc, st * 128 : (st + 1) * 128],
                                     qT[:, dc, :], start=(dc == 0), stop=(dc == N_DC - 1))
                return ss

            qT_cur = emit_qproj(0)
            for qb in range(N_QB):
                qT = qT_cur
                l_sb = rlp.tile([128, 512], F32, tag="l_sb", name=f"lsb{qb}")
                # at[ct][c in chunk, q] accumulates (P x)^T for this query
                # block: at[c, q] = sum_k x[k, c] pT[k, q]
                at_ps = [ps_atp.tile([128, 512], F32, tag="at", name=f"at{qb}_{ct}")
                         for ct in range(4)]
                ss_q = [emit_score(qb, 0, qT), emit_score(qb, 1, qT)]
                for st in range(N_ST):
                    if st + 2 < N_ST:
                        ss_q.append(emit_score(qb, st + 2, qT))
                    ss = ss_q.pop(0)
                    pT = pTp.tile([128, 512], MM_DT, tag="pT", name=f"pT{qb}_{st}")
                    nc.scalar.activation(pT[:], ss[:], ActFn.Exp, scale=SCALE)
                    for ct in range(4):
                        nc.tensor.matmul(at_ps[ct][:], xn[:, st, ct * 128 : (ct + 1) * 128],
                                         pT[:], start=(st == 0), stop=(st == N_ST - 1))
                    # row-sum accumulation on the DVE (off the PE)
                    if st == 0:
                        nc.vector.tensor_copy(l_sb[:], pT[:])
                    else:
                        nc.vector.tensor_add(l_sb[:], l_sb[:], pT[:])

                if qb + 1 < N_QB:
                    qT_cur = emit_qproj(qb + 1)

                # --- epilogue: 1/l arranged with queries on partitions.
                # One tiny bf16 matmul per 128-query block folds the partition
                # sum AND the transpose: lt[q, 0] = sum_p lbf[p, qt*128 + q] ---
                lbf = rlp.tile([128, 512], MM_DT, tag="lbf", name=f"lbf{qb}")
                nc.vector.tensor_copy(lbf[:], l_sb[:])
                lt_ps = ps_lp.tile([128, 4], F32, tag="l", name=f"lt{qb}")
                for qt in range(4):
                    nc.tensor.matmul(lt_ps[:, qt : qt + 1],
                                     lbf[:, qt * 128 : (qt + 1) * 128],
                                     ones_bf[:, 0:1])
                rlT = rlp.tile([128, 4], F32, tag="rlT", name=f"rlT{qb}")
                nc.vector.reciprocal(rlT[:], lt_ps[:])

                # (P x) to SBUF bf16, then apply N per 128-query block:
                # po[q, d] = sum_ct atb[:, ct, qt]^T N[:, ct, :], qt-outer so
                # po[0]'s scale + DMA overlap the remaining matmuls
                last = qb == N_QB - 1
                atb = atbp.tile([128, 4, 512], MM_DT, tag="atb", name=f"atb{qb}")
                for ct in range(4):
                    # for the final block split the copies across Scalar+Vector
                    # (no next-block exps to yield to), shortening the tail
                    if last and ct % 2:
                        nc.vector.tensor_copy(atb[:, ct, :], at_ps[ct][:])
                    else:
                        nc.scalar.activation(atb[:, ct, :], at_ps[ct][:], ActFn.Copy)
                for qt in range(4):
                    po = ps_atp.tile([128, 512], F32, tag="at", name=f"po{qb}_{qt}")
                    for ct in range(4):
                        nc.tensor.matmul(po[:], atb[:, ct, qt * 128 : (qt + 1) * 128],
                                         wts["N"][:, ct, :], start=(ct == 0), stop=(ct == 3))
                    ot = osbp.tile([128, D], MM_DT, tag="ot", name=f"ot{qb}_{qt}")
                    row = out[(qb * 4 + qt) * 128 : (qb * 4 + qt + 1) * 128, :]
                    if last:
                        nc.vector.tensor_scalar_mul(ot[:, 0:256], po[:, 0:256],
                                                    rlT[:, qt : qt + 1])
                        nc.scalar.activation(ot[:, 256:512], po[:, 256:512],
                                             ActFn.Copy, scale=rlT[:, qt : qt + 1])
                        eng1 = nc.scalar if qt < 2 else nc.sync
                        nc.sync.dma_start(row[:, 0:256], ot[:, 0:256])
                        eng1.dma_start(row[:, 256:512], ot[:, 256:512])
                    else:
                        nc.vector.tensor_scalar_mul(ot[:], po[:], rlT[:, qt : qt + 1])
                        eng = nc.sync if qt % 2 == 0 else nc.scalar
                        eng.dma_start(row, ot[:])

        const.release()
        persist.release()

    nc.compile()
    return nc


_NC_CACHE = None


def _get_nc():
    global _NC_CACHE
    if _NC_CACHE is None:
        _NC_CACHE = _build_program()
    return _NC_CACHE


def kernel(**inputs):
    f32 = np.float32
    x = np.asarray(inputs["x"], dtype=f32)
    xb16 = x.reshape(B, S, C).astype(ml_dtypes.bfloat16)
    # host-side transpose to xT[c, s] per batch
    xt = np.ascontiguousarray(xb16.transpose(0, 2, 1))

    def tile_x(xTb):
        # xT[c, s] -> [rg, dc, p, s] so each (rg, dc) chunk is contiguous
        return np.ascontiguousarray(
            xTb.reshape(N_DC, 128, N_RG, 512).transpose(2, 0, 1, 3))

    Wq = np.asarray(inputs["Wq"], dtype=f32)
    Wk = np.asarray(inputs["Wk"], dtype=f32)
    Wv = np.asarray(inputs["Wv"], dtype=f32)
    Wo = np.asarray(inputs["Wo"], dtype=f32)
    # host-side weight folds (exact algebra, f32):
    #   score = [x (Wq Wk^T) + bq Wk^T] x^T  (+ per-query const, cancels)
    #   out   = ((P x)/l) (Wv Wo) + (bv Wo + bo)
    M = np.ascontiguousarray(Wq @ Wk.T).astype(ml_dtypes.bfloat16)
    N = np.ascontiguousarray(Wv @ Wo).astype(ml_dtypes.bfloat16)
    bqf = np.ascontiguousarray(np.asarray(inputs["bq"], dtype=f32) @ Wk.T)
    bo_eff = np.asarray(inputs["bo"], dtype=f32) + np.asarray(inputs["bv"], dtype=f32) @ Wo

    in_maps = []
    for c in range(N_CORES):
        b, h = divmod(c, 2)
        # roll keys so this core's query half occupies rows 0..2047;
        # attention over keys is order-invariant so this is exact.
        if h:
            xnb = np.ascontiguousarray(
                np.concatenate([xb16[b][Q:], xb16[b][:Q]], axis=0))
            xTb = np.concatenate([xt[b][:, Q:], xt[b][:, :Q]], axis=1)
        else:
            xnb = np.ascontiguousarray(xb16[b])
            xTb = xt[b]
        in_maps.append({"x": tile_x(xTb), "xn": xnb, "bq": bqf, "M": M, "N": N})

    nc = _get_nc()
    try:
        res = run_bass_kernel_spmd(nc, in_maps, core_ids=list(range(N_CORES)))
    except Exception:
        # transient NRT/device hiccups recover on retry
        import time
        time.sleep(15)
        res = run_bass_kernel_spmd(nc, in_maps, core_ids=list(range(N_CORES)))

    out = np.empty((B, S, D), dtype=f32)
    for c in range(N_CORES):
        b, h = divmod(c, 2)
        out[b, h * Q : (h + 1) * Q] = np.asarray(res.results[c]["out"]).astype(f32)
    if np.any(bo_eff):
        out += bo_eff
    return out.reshape(B, 64, 64, D)


# revision 37
# speedup vs baseline: 1.0650x; 1.0650x over previous
"""Trainium2 Bass kernel for single-head attention (B=4, S=4096, C=D=512).

Sharding: 8 cores = 4 batches x 2 query-halves. Each core receives x in
BOTH orientations (host-pre-transposed xT tiles for contractions over
channels, and row-major xn tiles for the P@x contraction over keys),
ROLLED so its query half occupies rows 0..2047 (attention over keys is
order-invariant, so rolling keys is exact).

Both weight pairs fold on the host, and BOTH remaining projections are
reassociated to the query side (exact algebra, not an approximation):

  score_qk = (x_q Wq + bq)(x_k Wk + bk)^T
           = [x_q (Wq Wk^T) + bq Wk^T] x_k^T + const(q)   [cancels in softmax]
  out      = (P/l)(x Wv + bv) Wo + bo
           = ((P x)/l) (Wv Wo) + (bv Wo + bo)              [reassociated]

With M = Wq Wk^T and N = Wv Wo precomputed f32 on the host:
  - NO K projection: score matmuls contract q~ = x M + bq Wk^T directly
    against the resident xT chunks,
  - NO V projection over the 4096 keys (which was the one piece of
    work duplicated across a core pair): the attention accumulation
    computes at'[c, q] = sum_k x[k, c] P[q, k] with lhsT = xn key-tile
    slices, and N is applied afterwards per 512-query block
    (po[q, d] = sum_c at'[c, q] N[c, d]) -- query-side work only,
  - bq enters via the q~ copy's bias port, bk cancels in softmax, bv+bo
    are added on the host after gather.

Per-core PE work: ~247us of matmul streaming (scores 109 + P@x 109 +
q~ 13.7 + N-apply 13.7 + l-fold ~1).

On-chip layout notes:
  - xT is host-tiled ([rg, dc, p, s], dense 128KB chunks); xn row-groups
    DMA via the "(j p) c -> p j c" rearrange; both are the persistent
    matmul operands (no on-chip transposes or copies).
  - Scores are computed transposed (scoreT[s, q]) so exp(scoreT) feeds
    the P@x matmuls directly (as rhs) with no per-tile transposes.
  - Row sums l[q] accumulate on the DVE; one tiny bf16 matmul per
    128-query block folds the partition sum AND the transpose of l;
    1/l is applied via a per-partition scale AP.
  - The s-loop is software-pipelined: score matmuls for key-tile st+1/st+2
    are issued before the exp(st)-consuming matmuls so the in-order PE
    never waits on the ScalarE.
  - bf16 output DMA (adds <= 2^-8 relative rounding; metric stays ~1e-2
    << the 2e-2 gate) halves output traffic.
"""

import sys

for _p in ("/opt/trn_rl_repo", "/root/.axon_site/_ro/trn_rl_repo"):
    if _p not in sys.path:
        sys.path.append(_p)

import numpy as np
import ml_dtypes
import concourse.bacc as bacc
import concourse.mybir as mybir
import concourse.tile as tile
from concourse.bass_utils import run_bass_kernel_spmd

F32 = mybir.dt.float32
BF16 = mybir.dt.bfloat16

MM_DT = BF16

B, S, C, D = 4, 4096, 512, 512
Q = S // 2          # queries per core
N_CORES = 8
SCALE = float(D) ** -0.5
QB = 512            # query block (psum bank width in fp32)
N_QB = Q // QB      # 4 query blocks per core
N_ST = S // 128     # 32 key tiles
N_DC = C // 128     # 4 contraction chunks
N_RG = S // 512     # 8 row groups


def _build_program():
    nc = bacc.Bacc(None, target_bir_lowering=False, debug=False)

    # host-transposed AND host-tiled: x[rg, dc, p, s] = xT[dc*128+p, rg*512+s]
    x = nc.dram_tensor("x", [N_RG, N_DC, 128, 512], BF16, kind="ExternalInput")
    xn_dram = nc.dram_tensor("xn", [S, C], BF16, kind="ExternalInput")  # row-major
    w_dram = {
        name: nc.dram_tensor(name, [C, D], BF16, kind="ExternalInput")
        for name in ("M", "N")
    }
    bq_dram = nc.dram_tensor("bq", [D], F32, kind="ExternalInput")  # bq Wk^T
    out = nc.dram_tensor("out", [Q, D], BF16, kind="ExternalOutput")

    ActFn = mybir.ActivationFunctionType

    with tile.TileContext(nc) as tc:
        persist = tc.alloc_tile_pool(name="persist", bufs=1)
        const = tc.alloc_tile_pool(name="const", bufs=1)

        ones_f32 = const.tile([128, 128], F32, tag="ones_f32")
        nc.vector.memset(ones_f32[:], 1.0)
        ones_bf = const.tile([128, 1], MM_DT, tag="ones_bf")
        nc.vector.memset(ones_bf[:], 1.0)
        bqT = const.tile([128, N_DC], F32, tag="bqT")
        # tiny 4B-element bias DMAs, needed by the very first q~ copy
        for g in range(N_DC):
            nc.gpsimd.dma_start(bqT[:, g : g + 1],
                                bq_dram[g * 128 : (g + 1) * 128].unsqueeze(1))

        wts = {}

        def emit_weight(name, engine):
            wt = persist.tile([128, N_DC, D], MM_DT, tag=f"w_{name}", name=f"w_{name}")
            for dc in range(N_DC):
                engine.dma_start(wt[:, dc, :], w_dram[name][dc * 128 : (dc + 1) * 128, :])
            wts[name] = wt

        # ---- persistent activations ----
        xT = persist.tile([128, N_DC, S], BF16, tag="xT")   # xT[p, dc, s] = x[s, dc*128+p]
        xn = persist.tile([128, N_ST, C], BF16, tag="xn")   # xn[p, i, c] = x[i*128+p, c]

        def emit_xT(rg, eng):
            for dc in range(N_DC):
                eng.dma_start(xT[:, dc, rg * 512 : (rg + 1) * 512], x[rg, dc])

        def emit_xn(rg, eng):
            eng.dma_start(
                xn[:, rg * 4 : (rg + 1) * 4, :],
                xn_dram[rg * 512 : (rg + 1) * 512, :].rearrange("(j p) c -> p j c", j=4))

        # DMA order is deadline-driven: the q~ projection (M + xT rg0) gates
        # everything; then scores consume xT rgs and P@x consumes xn rgs in
        # lockstep during query-block 0.  Only the immediately-needed DMAs
        # are programmed up front: each DMA_DIRECT2D costs ~0.6us of ENGINE
        # time, and the scalar engine must get to the qT copies + exps fast.
        # The remaining bulk programming is spread inside query-block 0's
        # key loop (emitted between exps, see dma_hook below).
        emit_xT(0, nc.sync)
        emit_weight("M", nc.scalar)
        for j in range(2):              # first two key tiles, fine-grained
            nc.sync.dma_start(xn[:, j, :], xn_dram[j * 128 : (j + 1) * 128, :])
        for j in range(2, 4):
            nc.scalar.dma_start(xn[:, j, :], xn_dram[j * 128 : (j + 1) * 128, :])

        warm = const.tile([1, 1], F32, tag="warm")
        nc.scalar.activation(warm[:], ones_f32[0:1, 0:1], ActFn.Exp, scale=1.0)

        emit_xT(1, nc.sync)

        def dma_hook(st):
            # staged bulk DMA programming during query-block 0
            if st == 1:
                emit_xn(1, nc.scalar)   # key tiles 4-7, needed at st==4
            elif st == 2:
                emit_xT(2, nc.scalar)
            elif st == 3:
                emit_xn(2, nc.sync)
                emit_xn(3, nc.scalar)
            elif st == 7:
                emit_xT(3, nc.sync)
                emit_xT(4, nc.scalar)
            elif st == 11:
                emit_xn(4, nc.sync)
                emit_xn(5, nc.scalar)
            elif st == 15:
                emit_xT(5, nc.sync)
                emit_xT(6, nc.scalar)
            elif st == 19:
                emit_xn(6, nc.sync)
                emit_xn(7, nc.scalar)
            elif st == 23:
                emit_xT(7, nc.sync)
                emit_weight("N", nc.scalar)

        # ================= attention =================
        with tc.tile_pool(name="qT", bufs=2) as qTp, \
             tc.tile_pool(name="pT", bufs=8) as pTp, \
             tc.tile_pool(name="rl", bufs=2) as rlp, \
             tc.tile_pool(name="atb", bufs=2) as atbp, \
             tc.tile_pool(name="osb", bufs=4) as osbp, \
             tc.tile_pool(name="ps_at", bufs=4, space="PSUM") as ps_atp, \
             tc.tile_pool(name="ps_s", bufs=3, space="PSUM") as ps_sp, \
             tc.tile_pool(name="ps_l", bufs=1, space="PSUM") as ps_lp:

            def emit_qproj(qb):
                # q~ = x M + bq Wk^T for one 512-query block, straight from
                # the resident xT columns [qb*512, (qb+1)*512).
                qT = qTp.tile([128, N_DC, 512], MM_DT, tag="qT", name=f"qT{qb}")
                for g in range(N_DC):
                    pq = ps_sp.tile([128, 512], F32, tag="ss", name=f"pq{qb}_{g}")
                    for dc in range(N_DC):
                        nc.tensor.matmul(pq[:], wts["M"][:, dc, g * 128 : (g + 1) * 128],
                                         xT[:, dc, qb * 512 : (qb + 1) * 512],
                                         start=(dc == 0), stop=(dc == N_DC - 1))
                    nc.scalar.activation(qT[:, g, :], pq[:], ActFn.Identity,
                                         bias=bqT[:, g : g + 1])
                return qT

            def emit_score(qb, st, qT):
                # scoreT[s in st, q] = sum_dc xT[:, dc, st]^T qT[:, dc, :]
                ss = ps_sp.tile([128, 512], F32, tag="ss", name=f"ss{qb}_{st}")
                for dc in range(N_DC):
                    nc.tensor.matmul(ss[:], xT[:, dc, st * 128 : (st + 1) * 128],
                                     qT[:, dc, :], start=(dc == 0), stop=(dc == N_DC - 1))
                return ss

            qT_cur = emit_qproj(0)
            for qb in range(N_QB):
                qT = qT_cur
                l_sb = rlp.tile([128, 512], F32, tag="l_sb", name=f"lsb{qb}")
                # at[ct][c in chunk, q] accumulates (P x)^T for this query
                # block: at[c, q] = sum_k x[k, c] pT[k, q]
                at_ps = [ps_atp.tile([128, 512], F32, tag="at", name=f"at{qb}_{ct}")
                         for ct in range(4)]
                ss_q = [emit_score(qb, 0, qT), emit_score(qb, 1, qT)]
                for st in range(N_ST):
                    if qb == 0:
                        dma_hook(st)
                    if st + 2 < N_ST:
                        ss_q.append(emit_score(qb, st + 2, qT))
                    ss = ss_q.pop(0)
                    pT = pTp.tile([128, 512], MM_DT, tag="pT", name=f"pT{qb}_{st}")
                    nc.scalar.activation(pT[:], ss[:], ActFn.Exp, scale=SCALE)
                    for ct in range(4):
                        nc.tensor.matmul(at_ps[ct][:], xn[:, st, ct * 128 : (ct + 1) * 128],
                                         pT[:], start=(st == 0), stop=(st == N_ST - 1))
                    # row-sum accumulation on the DVE (off the PE)
                    if st == 0:
                        nc.vector.tensor_copy(l_sb[:], pT[:])
                    else:
                        nc.vector.tensor_add(l_sb[:], l_sb[:], pT[:])

                if qb + 1 < N_QB:
                    qT_cur = emit_qproj(qb + 1)

                # --- epilogue: 1/l arranged with queries on partitions.
                # One tiny bf16 matmul per 128-query block folds the partition
                # sum AND the transpose: lt[q, 0] = sum_p lbf[p, qt*128 + q] ---
                lbf = rlp.tile([128, 512], MM_DT, tag="lbf", name=f"lbf{qb}")
                nc.vector.tensor_copy(lbf[:], l_sb[:])
                lt_ps = ps_lp.tile([128, 4], F32, tag="l", name=f"lt{qb}")
                for qt in range(4):
                    nc.tensor.matmul(lt_ps[:, qt : qt + 1],
                                     lbf[:, qt * 128 : (qt + 1) * 128],
                                     ones_bf[:, 0:1])
                rlT = rlp.tile([128, 4], F32, tag="rlT", name=f"rlT{qb}")
                nc.vector.reciprocal(rlT[:], lt_ps[:])

                # (P x) to SBUF bf16, then apply N per 128-query block:
                # po[q, d] = sum_ct atb[:, ct, qt]^T N[:, ct, :], qt-outer so
                # po[0]'s scale + DMA overlap the remaining matmuls
                last = qb == N_QB - 1
                atb = atbp.tile([128, 4, 512], MM_DT, tag="atb", name=f"atb{qb}")
                for ct in range(4):
                    # for the final block split the copies across Scalar+Vector
                    # (no next-block exps to yield to), shortening the tail
                    if last and ct % 2:
                        nc.vector.tensor_copy(atb[:, ct, :], at_ps[ct][:])
                    else:
                        nc.scalar.activation(atb[:, ct, :], at_ps[ct][:], ActFn.Copy)
                for qt in range(4):
                    po = ps_atp.tile([128, 512], F32, tag="at", name=f"po{qb}_{qt}")
                    for ct in range(4):
                        nc.tensor.matmul(po[:], atb[:, ct, qt * 128 : (qt + 1) * 128],
                                         wts["N"][:, ct, :], start=(ct == 0), stop=(ct == 3))
                    ot = osbp.tile([128, D], MM_DT, tag="ot", name=f"ot{qb}_{qt}")
                    row = out[(qb * 4 + qt) * 128 : (qb * 4 + qt + 1) * 128, :]
                    if last:
                        nc.vector.tensor_scalar_mul(ot[:, 0:256], po[:, 0:256],
                                                    rlT[:, qt : qt + 1])
                        nc.scalar.activation(ot[:, 256:512], po[:, 256:512],
                                             ActFn.Copy, scale=rlT[:, qt : qt + 1])
                        eng1 = nc.scalar if qt < 2 else nc.sync
                        nc.sync.dma_start(row[:, 0:256], ot[:, 0:256])
                        eng1.dma_start(row[:, 256:512], ot[:, 256:512])
                    else:
                        nc.vector.tensor_scalar_mul(ot[:], po[:], rlT[:, qt : qt + 1])
                        eng = nc.sync if qt % 2 == 0 else nc.scalar
                        eng.dma_start(row, ot[:])

        const.release()
        persist.release()

    nc.compile()
    return nc


_NC_CACHE = None


def _get_nc():
    global _NC_CACHE
    if _NC_CACHE is None:
        _NC_CACHE = _build_program()
    return _NC_CACHE


def kernel(**inputs):
    f32 = np.float32
    x = np.asarray(inputs["x"], dtype=f32)
    xb16 = x.reshape(B, S, C).astype(ml_dtypes.bfloat16)
    # host-side transpose to xT[c, s] per batch
    xt = np.ascontiguousarray(xb16.transpose(0, 2, 1))

    def tile_x(xTb):
        # xT[c, s] -> [rg, dc, p, s] so each (rg, dc) chunk is contiguous
        return np.ascontiguousarray(
            xTb.reshape(N_DC, 128, N_RG, 512).transpose(2, 0, 1, 3))

    Wq = np.asarray(inputs["Wq"], dtype=f32)
    Wk = np.asarray(inputs["Wk"], dtype=f32)
    Wv = np.asarray(inputs["Wv"], dtype=f32)
    Wo = np.asarray(inputs["Wo"], dtype=f32)
    # host-side weight folds (exact algebra, f32):
    #   score = [x (Wq Wk^T) + bq Wk^T] x^T  (+ per-query const, cancels)
    #   out   = ((P x)/l) (Wv Wo) + (bv Wo + bo)
    M = np.ascontiguousarray(Wq @ Wk.T).astype(ml_dtypes.bfloat16)
    N = np.ascontiguousarray(Wv @ Wo).astype(ml_dtypes.bfloat16)
    bqf = np.ascontiguousarray(np.asarray(inputs["bq"], dtype=f32) @ Wk.T)
    bo_eff = np.asarray(inputs["bo"], dtype=f32) + np.asarray(inputs["bv"], dtype=f32) @ Wo

    in_maps = []
    for c in range(N_CORES):
        b, h = divmod(c, 2)
        # roll keys so this core's query half occupies rows 0..2047;
        # attention over keys is order-invariant so this is exact.
        if h:
            xnb = np.ascontiguousarray(
                np.concatenate([xb16[b][Q:], xb16[b][:Q]], axis=0))
            xTb = np.concatenate([xt[b][:, Q:], xt[b][:, :Q]], axis=1)
        else:
            xnb = np.ascontiguousarray(xb16[b])
            xTb = xt[b]
        in_maps.append({"x": tile_x(xTb), "xn": xnb, "bq": bqf, "M": M, "N": N})

    nc = _get_nc()
    try:
        res = run_bass_kernel_spmd(nc, in_maps, core_ids=list(range(N_CORES)))
    except Exception:
        # transient NRT/device hiccups recover on retry
        import time
        time.sleep(15)
        res = run_bass_kernel_spmd(nc, in_maps, core_ids=list(range(N_CORES)))

    out = np.empty((B, S, D), dtype=f32)
    for c in range(N_CORES):
        b, h = divmod(c, 2)
        out[b, h * Q : (h + 1) * Q] = np.asarray(res.results[c]["out"]).astype(f32)
    if np.any(bo_eff):
        out += bo_eff
    return out.reshape(B, 64, 64, D)


# revision 39
# speedup vs baseline: 1.0769x; 1.0112x over previous
"""Trainium2 Bass kernel for single-head attention (B=4, S=4096, C=D=512).

Sharding: 8 cores = 4 batches x 2 query-halves. Each core receives x in
BOTH orientations (host-pre-transposed xT tiles for contractions over
channels, and row-major xn tiles for the P@x contraction over keys),
ROLLED so its query half occupies rows 0..2047 (attention over keys is
order-invariant, so rolling keys is exact).

Both weight pairs fold on the host, and BOTH remaining projections are
reassociated to the query side (exact algebra, not an approximation):

  score_qk = (x_q Wq + bq)(x_k Wk + bk)^T
           = [x_q (Wq Wk^T) + bq Wk^T] x_k^T + const(q)   [cancels in softmax]
  out      = (P/l)(x Wv + bv) Wo + bo
           = ((P x)/l) (Wv Wo) + (bv Wo + bo)              [reassociated]

With M = Wq Wk^T and N = Wv Wo precomputed f32 on the host:
  - NO K projection: score matmuls contract q~ = x M + bq Wk^T directly
    against the resident xT chunks,
  - NO V projection over the 4096 keys (which was the one piece of
    work duplicated across a core pair): the attention accumulation
    computes at'[c, q] = sum_k x[k, c] P[q, k] with lhsT = xn key-tile
    slices, and N is applied afterwards per 512-query block
    (po[q, d] = sum_c at'[c, q] N[c, d]) -- query-side work only,
  - bq enters via the q~ copy's bias port, bk cancels in softmax, bv+bo
    are added on the host after gather.

Per-core PE work: ~247us of matmul streaming (scores 109 + P@x 109 +
q~ 13.7 + N-apply 13.7 + l-fold ~1).

On-chip layout notes:
  - xT is host-tiled ([rg, dc, p, s], dense 128KB chunks); xn row-groups
    DMA via the "(j p) c -> p j c" rearrange; both are the persistent
    matmul operands (no on-chip transposes or copies).
  - Scores are computed transposed (scoreT[s, q]) so exp(scoreT) feeds
    the P@x matmuls directly (as rhs) with no per-tile transposes.
  - Row sums l[q] accumulate on the DVE; one tiny bf16 matmul per
    128-query block folds the partition sum AND the transpose of l;
    1/l is applied via a per-partition scale AP.
  - The s-loop is software-pipelined: score matmuls for key-tile st+1/st+2
    are issued before the exp(st)-consuming matmuls so the in-order PE
    never waits on the ScalarE.
  - bf16 output DMA (adds <= 2^-8 relative rounding; metric stays ~1e-2
    << the 2e-2 gate) halves output traffic.
"""

import sys

for _p in ("/opt/trn_rl_repo", "/root/.axon_site/_ro/trn_rl_repo"):
    if _p not in sys.path:
        sys.path.append(_p)

import numpy as np
import ml_dtypes
import concourse.bacc as bacc
import concourse.mybir as mybir
import concourse.tile as tile
from concourse.bass_utils import run_bass_kernel_spmd

F32 = mybir.dt.float32
BF16 = mybir.dt.bfloat16

MM_DT = BF16

B, S, C, D = 4, 4096, 512, 512
Q = S // 2          # queries per core
N_CORES = 8
SCALE = float(D) ** -0.5
QB = 512            # query block (psum bank width in fp32)
N_QB = Q // QB      # 4 query blocks per core
N_ST = S // 128     # 32 key tiles
N_DC = C // 128     # 4 contraction chunks
N_RG = S // 512     # 8 row groups


def _build_program():
    nc = bacc.Bacc(None, target_bir_lowering=False, debug=False)

    # host-transposed AND host-tiled: x[rg, dc, p, s] = xT[dc*128+p, rg*512+s]
    x = nc.dram_tensor("x", [N_RG, N_DC, 128, 512], BF16, kind="ExternalInput")
    xn_dram = nc.dram_tensor("xn", [S, C], BF16, kind="ExternalInput")  # row-major
    w_dram = {
        name: nc.dram_tensor(name, [C, D], BF16, kind="ExternalInput")
        for name in ("M", "N")
    }
    bq_dram = nc.dram_tensor("bq", [D], F32, kind="ExternalInput")  # bq Wk^T
    out = nc.dram_tensor("out", [Q, D], BF16, kind="ExternalOutput")

    ActFn = mybir.ActivationFunctionType

    with tile.TileContext(nc) as tc:
        persist = tc.alloc_tile_pool(name="persist", bufs=1)
        const = tc.alloc_tile_pool(name="const", bufs=1)

        ones_f32 = const.tile([128, 128], F32, tag="ones_f32")
        nc.vector.memset(ones_f32[:], 1.0)
        ones_bf = const.tile([128, 1], MM_DT, tag="ones_bf")
        nc.vector.memset(ones_bf[:], 1.0)
        bqT = const.tile([128, N_DC], F32, tag="bqT")
        # tiny 4B-element bias DMAs, needed by the very first q~ copy
        for g in range(N_DC):
            nc.gpsimd.dma_start(bqT[:, g : g + 1],
                                bq_dram[g * 128 : (g + 1) * 128].unsqueeze(1))

        wts = {}

        def emit_weight(name, engine):
            wt = persist.tile([128, N_DC, D], MM_DT, tag=f"w_{name}", name=f"w_{name}")
            for dc in range(N_DC):
                engine.dma_start(wt[:, dc, :], w_dram[name][dc * 128 : (dc + 1) * 128, :])
            wts[name] = wt

        # ---- persistent activations ----
        xT = persist.tile([128, N_DC, S], BF16, tag="xT")   # xT[p, dc, s] = x[s, dc*128+p]
        xn = persist.tile([128, N_ST, C], BF16, tag="xn")   # xn[p, i, c] = x[i*128+p, c]

        def emit_xT(rg, eng):
            for dc in range(N_DC):
                eng.dma_start(xT[:, dc, rg * 512 : (rg + 1) * 512], x[rg, dc])

        def emit_xn(rg, eng):
            eng.dma_start(
                xn[:, rg * 4 : (rg + 1) * 4, :],
                xn_dram[rg * 512 : (rg + 1) * 512, :].rearrange("(j p) c -> p j c", j=4))

        # DMA order is deadline-driven: the q~ projection (M + xT rg0) gates
        # everything; then scores consume xT rgs and P@x consumes xn rgs in
        # lockstep during query-block 0.  Only the immediately-needed DMAs
        # are programmed up front: each DMA_DIRECT2D costs ~0.6us of ENGINE
        # time, and the scalar engine must get to the qT copies + exps fast.
        # The remaining bulk programming is spread inside query-block 0's
        # key loop (emitted between exps, see dma_hook below).
        emit_xT(0, nc.sync)
        emit_weight("M", nc.scalar)
        for j in range(2):              # first two key tiles, fine-grained
            nc.sync.dma_start(xn[:, j, :], xn_dram[j * 128 : (j + 1) * 128, :])
        for j in range(2, 4):
            nc.scalar.dma_start(xn[:, j, :], xn_dram[j * 128 : (j + 1) * 128, :])
        emit_xn(1, nc.scalar)           # key tiles 4-7, needed at st==4

        warm = const.tile([1, 1], F32, tag="warm")
        nc.scalar.activation(warm[:], ones_f32[0:1, 0:1], ActFn.Exp, scale=1.0)

        emit_xT(1, nc.sync)

        def dma_hook(st):
            # staged bulk DMA programming during query-block 0
            if st == 2:
                emit_xT(2, nc.scalar)
            elif st == 3:
                emit_xn(2, nc.sync)
                emit_xn(3, nc.scalar)
            elif st == 7:
                emit_xT(3, nc.sync)
                emit_xT(4, nc.scalar)
            elif st == 11:
                emit_xn(4, nc.sync)
                emit_xn(5, nc.scalar)
            elif st == 15:
                emit_xT(5, nc.sync)
                emit_xT(6, nc.scalar)
            elif st == 19:
                emit_xn(6, nc.sync)
                emit_xn(7, nc.scalar)
            elif st == 23:
                emit_xT(7, nc.sync)
                emit_weight("N", nc.scalar)

        # ================= attention =================
        with tc.tile_pool(name="qT", bufs=2) as qTp, \
             tc.tile_pool(name="pT", bufs=8) as pTp, \
             tc.tile_pool(name="rl", bufs=2) as rlp, \
             tc.tile_pool(name="atb", bufs=2) as atbp, \
             tc.tile_pool(name="osb", bufs=4) as osbp, \
             tc.tile_pool(name="ps_at", bufs=4, space="PSUM") as ps_atp, \
             tc.tile_pool(name="ps_s", bufs=3, space="PSUM") as ps_sp, \
             tc.tile_pool(name="ps_l", bufs=1, space="PSUM") as ps_lp:

            def emit_qproj(qb):
                # q~ = x M + bq Wk^T for one 512-query block, straight from
                # the resident xT columns [qb*512, (qb+1)*512).
                qT = qTp.tile([128, N_DC, 512], MM_DT, tag="qT", name=f"qT{qb}")
                for g in range(N_DC):
                    pq = ps_sp.tile([128, 512], F32, tag="ss", name=f"pq{qb}_{g}")
                    for dc in range(N_DC):
                        nc.tensor.matmul(pq[:], wts["M"][:, dc, g * 128 : (g + 1) * 128],
                                         xT[:, dc, qb * 512 : (qb + 1) * 512],
                                         start=(dc == 0), stop=(dc == N_DC - 1))
                    nc.scalar.activation(qT[:, g, :], pq[:], ActFn.Identity,
                                         bias=bqT[:, g : g + 1])
                return qT

            def emit_score(qb, st, qT):
                # scoreT[s in st, q] = sum_dc xT[:, dc, st]^T qT[:, dc, :]
                ss = ps_sp.tile([128, 512], F32, tag="ss", name=f"ss{qb}_{st}")
                for dc in range(N_DC):
                    nc.tensor.matmul(ss[:], xT[:, dc, st * 128 : (st + 1) * 128],
                                     qT[:, dc, :], start=(dc == 0), stop=(dc == N_DC - 1))
                return ss

            qT_cur = emit_qproj(0)
            for qb in range(N_QB):
                qT = qT_cur
                l_sb = rlp.tile([128, 512], F32, tag="l_sb", name=f"lsb{qb}")
                # at[ct][c in chunk, q] accumulates (P x)^T for this query
                # block: at[c, q] = sum_k x[k, c] pT[k, q]
                at_ps = [ps_atp.tile([128, 512], F32, tag="at", name=f"at{qb}_{ct}")
                         for ct in range(4)]
                ss_q = [emit_score(qb, 0, qT), emit_score(qb, 1, qT)]
                for st in range(N_ST):
                    if qb == 0:
                        dma_hook(st)
                    if st + 2 < N_ST:
                        ss_q.append(emit_score(qb, st + 2, qT))
                    ss = ss_q.pop(0)
                    pT = pTp.tile([128, 512], MM_DT, tag="pT", name=f"pT{qb}_{st}")
                    nc.scalar.activation(pT[:], ss[:], ActFn.Exp, scale=SCALE)
                    for ct in range(4):
                        nc.tensor.matmul(at_ps[ct][:], xn[:, st, ct * 128 : (ct + 1) * 128],
                                         pT[:], start=(st == 0), stop=(st == N_ST - 1))
                    # row-sum accumulation on the DVE (off the PE)
                    if st == 0:
                        nc.vector.tensor_copy(l_sb[:], pT[:])
                    else:
                        nc.vector.tensor_add(l_sb[:], l_sb[:], pT[:])

                if qb + 1 < N_QB:
                    qT_cur = emit_qproj(qb + 1)

                # --- epilogue: 1/l arranged with queries on partitions.
                # One tiny bf16 matmul per 128-query block folds the partition
                # sum AND the transpose: lt[q, 0] = sum_p lbf[p, qt*128 + q] ---
                lbf = rlp.tile([128, 512], MM_DT, tag="lbf", name=f"lbf{qb}")
                nc.vector.tensor_copy(lbf[:], l_sb[:])
                lt_ps = ps_lp.tile([128, 4], F32, tag="l", name=f"lt{qb}")
                for qt in range(4):
                    nc.tensor.matmul(lt_ps[:, qt : qt + 1],
                                     lbf[:, qt * 128 : (qt + 1) * 128],
                                     ones_bf[:, 0:1])
                rlT = rlp.tile([128, 4], F32, tag="rlT", name=f"rlT{qb}")
                nc.vector.reciprocal(rlT[:], lt_ps[:])

                # (P x) to SBUF bf16, then apply N per 128-query block:
                # po[q, d] = sum_ct atb[:, ct, qt]^T N[:, ct, :], qt-outer so
                # po[0]'s scale + DMA overlap the remaining matmuls
                last = qb == N_QB - 1
                atb = atbp.tile([128, 4, 512], MM_DT, tag="atb", name=f"atb{qb}")
                for ct in range(4):
                    # for the final block split the copies across Scalar+Vector
                    # (no next-block exps to yield to), shortening the tail
                    if last and ct % 2:
                        nc.vector.tensor_copy(atb[:, ct, :], at_ps[ct][:])
                    else:
                        nc.scalar.activation(atb[:, ct, :], at_ps[ct][:], ActFn.Copy)
                for qt in range(4):
                    po = ps_atp.tile([128, 512], F32, tag="at", name=f"po{qb}_{qt}")
                    for ct in range(4):
                        nc.tensor.matmul(po[:], atb[:, ct, qt * 128 : (qt + 1) * 128],
                                         wts["N"][:, ct, :], start=(ct == 0), stop=(ct == 3))
                    ot = osbp.tile([128, D], MM_DT, tag="ot", name=f"ot{qb}_{qt}")
                    row = out[(qb * 4 + qt) * 128 : (qb * 4 + qt + 1) * 128, :]
                    if last:
                        nc.vector.tensor_scalar_mul(ot[:, 0:256], po[:, 0:256],
                                                    rlT[:, qt : qt + 1])
                        nc.scalar.activation(ot[:, 256:512], po[:, 256:512],
                                             ActFn.Copy, scale=rlT[:, qt : qt + 1])
                        eng1 = nc.scalar if qt < 2 else nc.sync
                        nc.sync.dma_start(row[:, 0:256], ot[:, 0:256])
                        eng1.dma_start(row[:, 256:512], ot[:, 256:512])
                    else:
                        nc.vector.tensor_scalar_mul(ot[:], po[:], rlT[:, qt : qt + 1])
                        eng = nc.sync if qt % 2 == 0 else nc.scalar
                        eng.dma_start(row, ot[:])

        const.release()
        persist.release()

    nc.compile()
    return nc


_NC_CACHE = None


def _get_nc():
    global _NC_CACHE
    if _NC_CACHE is None:
        _NC_CACHE = _build_program()
    return _NC_CACHE


def kernel(**inputs):
    f32 = np.float32
    x = np.asarray(inputs["x"], dtype=f32)
    xb16 = x.reshape(B, S, C).astype(ml_dtypes.bfloat16)
    # host-side transpose to xT[c, s] per batch
    xt = np.ascontiguousarray(xb16.transpose(0, 2, 1))

    def tile_x(xTb):
        # xT[c, s] -> [rg, dc, p, s] so each (rg, dc) chunk is contiguous
        return np.ascontiguousarray(
            xTb.reshape(N_DC, 128, N_RG, 512).transpose(2, 0, 1, 3))

    Wq = np.asarray(inputs["Wq"], dtype=f32)
    Wk = np.asarray(inputs["Wk"], dtype=f32)
    Wv = np.asarray(inputs["Wv"], dtype=f32)
    Wo = np.asarray(inputs["Wo"], dtype=f32)
    # host-side weight folds (exact algebra, f32):
    #   score = [x (Wq Wk^T) + bq Wk^T] x^T  (+ per-query const, cancels)
    #   out   = ((P x)/l) (Wv Wo) + (bv Wo + bo)
    M = np.ascontiguousarray(Wq @ Wk.T).astype(ml_dtypes.bfloat16)
    N = np.ascontiguousarray(Wv @ Wo).astype(ml_dtypes.bfloat16)
    bqf = np.ascontiguousarray(np.asarray(inputs["bq"], dtype=f32) @ Wk.T)
    bo_eff = np.asarray(inputs["bo"], dtype=f32) + np.asarray(inputs["bv"], dtype=f32) @ Wo

    in_maps = []
    for c in range(N_CORES):
        b, h = divmod(c, 2)
        # roll keys so this core's query half occupies rows 0..2047;
        # attention over keys is order-invariant so this is exact.
        if h:
            xnb = np.ascontiguousarray(
                np.concatenate([xb16[b][Q:], xb16[b][:Q]], axis=0))
            xTb = np.concatenate([xt[b][:, Q:], xt[b][:, :Q]], axis=1)
        else:
            xnb = np.ascontiguousarray(xb16[b])
            xTb = xt[b]
        in_maps.append({"x": tile_x(xTb), "xn": xnb, "bq": bqf, "M": M, "N": N})

    nc = _get_nc()
    try:
        res = run_bass_kernel_spmd(nc, in_maps, core_ids=list(range(N_CORES)))
    except Exception:
        # transient NRT/device hiccups recover on retry
        import time
        time.sleep(15)
        res = run_bass_kernel_spmd(nc, in_maps, core_ids=list(range(N_CORES)))

    out = np.empty((B, S, D), dtype=f32)
    for c in range(N_CORES):
        b, h = divmod(c, 2)
        out[b, h * Q : (h + 1) * Q] = np.asarray(res.results[c]["out"]).astype(f32)
    if np.any(bo_eff):
        out += bo_eff
    return out.reshape(B, 64, 64, D)


# revision 41
# speedup vs baseline: 1.0840x; 1.0066x over previous
"""Trainium2 Bass kernel for single-head attention (B=4, S=4096, C=D=512).

Sharding: 8 cores = 4 batches x 2 query-halves. Each core receives x in
BOTH orientations (host-pre-transposed xT tiles for contractions over
channels, and row-major xn tiles for the P@x contraction over keys),
ROLLED so its query half occupies rows 0..2047 (attention over keys is
order-invariant, so rolling keys is exact).

Both weight pairs fold on the host, and BOTH remaining projections are
reassociated to the query side (exact algebra, not an approximation):

  score_qk = (x_q Wq + bq)(x_k Wk + bk)^T
           = [x_q (Wq Wk^T) + bq Wk^T] x_k^T + const(q)   [cancels in softmax]
  out      = (P/l)(x Wv + bv) Wo + bo
           = ((P x)/l) (Wv Wo) + (bv Wo + bo)              [reassociated]

With M = Wq Wk^T and N = Wv Wo precomputed f32 on the host:
  - NO K projection: score matmuls contract q~ = x M + bq Wk^T directly
    against the resident xT chunks,
  - NO V projection over the 4096 keys (which was the one piece of
    work duplicated across a core pair): the attention accumulation
    computes at'[c, q] = sum_k x[k, c] P[q, k] with lhsT = xn key-tile
    slices, and N is applied afterwards per 512-query block
    (po[q, d] = sum_c at'[c, q] N[c, d]) -- query-side work only,
  - bq enters via the q~ copy's bias port, bk cancels in softmax, bv+bo
    are added on the host after gather.

Per-core PE work: ~247us of matmul streaming (scores 109 + P@x 109 +
q~ 13.7 + N-apply 13.7 + l-fold ~1).

On-chip layout notes:
  - xT is host-tiled ([rg, dc, p, s], dense 128KB chunks); xn row-groups
    DMA via the "(j p) c -> p j c" rearrange; both are the persistent
    matmul operands (no on-chip transposes or copies).
  - Scores are computed transposed (scoreT[s, q]) so exp(scoreT) feeds
    the P@x matmuls directly (as rhs) with no per-tile transposes.
  - Row sums l[q] accumulate on the DVE; one tiny bf16 matmul per
    128-query block folds the partition sum AND the transpose of l;
    1/l is applied via a per-partition scale AP.
  - The s-loop is software-pipelined: score matmuls for key-tile st+1/st+2
    are issued before the exp(st)-consuming matmuls so the in-order PE
    never waits on the ScalarE.
  - bf16 output DMA (adds <= 2^-8 relative rounding; metric stays ~1e-2
    << the 2e-2 gate) halves output traffic.
"""

import sys

for _p in ("/opt/trn_rl_repo", "/root/.axon_site/_ro/trn_rl_repo"):
    if _p not in sys.path:
        sys.path.append(_p)

import numpy as np
import ml_dtypes
import concourse.bacc as bacc
import concourse.mybir as mybir
import concourse.tile as tile
from concourse.bass_utils import run_bass_kernel_spmd

F32 = mybir.dt.float32
BF16 = mybir.dt.bfloat16

MM_DT = BF16

B, S, C, D = 4, 4096, 512, 512
Q = S // 2          # queries per core
N_CORES = 8
SCALE = float(D) ** -0.5
QB = 512            # query block (psum bank width in fp32)
N_QB = Q // QB      # 4 query blocks per core
N_ST = S // 128     # 32 key tiles
N_DC = C // 128     # 4 contraction chunks
N_RG = S // 512     # 8 row groups


def _build_program():
    nc = bacc.Bacc(None, target_bir_lowering=False, debug=False)

    # host-transposed AND host-tiled: x[rg, dc, p, s] = xT[dc*128+p, rg*512+s]
    x = nc.dram_tensor("x", [N_RG, N_DC, 128, 512], BF16, kind="ExternalInput")
    xn_dram = nc.dram_tensor("xn", [S, C], BF16, kind="ExternalInput")  # row-major
    w_dram = {
        name: nc.dram_tensor(name, [C, D], BF16, kind="ExternalInput")
        for name in ("M", "N")
    }
    bq_dram = nc.dram_tensor("bq", [D], F32, kind="ExternalInput")  # bq Wk^T
    out = nc.dram_tensor("out", [Q, D], BF16, kind="ExternalOutput")

    ActFn = mybir.ActivationFunctionType

    with tile.TileContext(nc) as tc:
        persist = tc.alloc_tile_pool(name="persist", bufs=1)
        const = tc.alloc_tile_pool(name="const", bufs=1)

        ones_f32 = const.tile([128, 128], F32, tag="ones_f32")
        nc.vector.memset(ones_f32[:], 1.0)
        ones_bf = const.tile([128, 1], MM_DT, tag="ones_bf")
        nc.vector.memset(ones_bf[:], 1.0)
        bqT = const.tile([128, N_DC], F32, tag="bqT")
        # tiny 4B-element bias DMAs, needed by the very first q~ copy
        for g in range(N_DC):
            nc.gpsimd.dma_start(bqT[:, g : g + 1],
                                bq_dram[g * 128 : (g + 1) * 128].unsqueeze(1))

        wts = {}

        def emit_weight(name, engine):
            wt = persist.tile([128, N_DC, D], MM_DT, tag=f"w_{name}", name=f"w_{name}")
            for dc in range(N_DC):
                engine.dma_start(wt[:, dc, :], w_dram[name][dc * 128 : (dc + 1) * 128, :])
            wts[name] = wt

        # ---- persistent activations ----
        xT = persist.tile([128, N_DC, S], BF16, tag="xT")   # xT[p, dc, s] = x[s, dc*128+p]
        xn = persist.tile([128, N_ST, C], BF16, tag="xn")   # xn[p, i, c] = x[i*128+p, c]

        def emit_xT(rg, eng):
            for dc in range(N_DC):
                eng.dma_start(xT[:, dc, rg * 512 : (rg + 1) * 512], x[rg, dc])

        def emit_xn(rg, eng):
            eng.dma_start(
                xn[:, rg * 4 : (rg + 1) * 4, :],
                xn_dram[rg * 512 : (rg + 1) * 512, :].rearrange("(j p) c -> p j c", j=4))

        # DMA order is deadline-driven: the q~ projection (M + xT rg0) gates
        # everything; then scores consume xT rgs and P@x consumes xn rgs in
        # lockstep during query-block 0.  Only the immediately-needed DMAs
        # are programmed up front: each DMA_DIRECT2D costs ~0.6us of ENGINE
        # time, and the scalar engine must get to the qT copies + exps fast.
        # The remaining bulk programming is spread inside query-block 0's
        # key loop (emitted between exps, see dma_hook below).
        # keep the scalar ENGINE's pre-copy backlog minimal (M + xn1 programs
        # only): the q~ copies gate the whole score pipeline.  The first four
        # key tiles ride sync (engine idle, queue has slack before st0).
        emit_xT(0, nc.sync)
        emit_weight("M", nc.scalar)
        for j in range(4):              # first four key tiles, fine-grained
            nc.sync.dma_start(xn[:, j, :], xn_dram[j * 128 : (j + 1) * 128, :])
        emit_xn(1, nc.scalar)           # key tiles 4-7, needed at st==4
        emit_xT(1, nc.sync)

        def dma_hook(st):
            # staged bulk DMA programming during query-block 0
            if st == 2:
                emit_xT(2, nc.scalar)
            elif st == 3:
                emit_xn(2, nc.sync)
                emit_xn(3, nc.scalar)
            elif st == 7:
                emit_xT(3, nc.sync)
                emit_xT(4, nc.scalar)
            elif st == 11:
                emit_xn(4, nc.sync)
                emit_xn(5, nc.scalar)
            elif st == 15:
                emit_xT(5, nc.sync)
                emit_xT(6, nc.scalar)
            elif st == 19:
                emit_xn(6, nc.sync)
                emit_xn(7, nc.scalar)
            elif st == 23:
                emit_xT(7, nc.sync)
                emit_weight("N", nc.scalar)

        # ================= attention =================
        with tc.tile_pool(name="qT", bufs=2) as qTp, \
             tc.tile_pool(name="pT", bufs=8) as pTp, \
             tc.tile_pool(name="rl", bufs=2) as rlp, \
             tc.tile_pool(name="atb", bufs=2) as atbp, \
             tc.tile_pool(name="osb", bufs=4) as osbp, \
             tc.tile_pool(name="ps_at", bufs=4, space="PSUM") as ps_atp, \
             tc.tile_pool(name="ps_s", bufs=3, space="PSUM") as ps_sp, \
             tc.tile_pool(name="ps_l", bufs=1, space="PSUM") as ps_lp:

            def emit_qproj(qb):
                # q~ = x M + bq Wk^T for one 512-query block, straight from
                # the resident xT columns [qb*512, (qb+1)*512).
                qT = qTp.tile([128, N_DC, 512], MM_DT, tag="qT", name=f"qT{qb}")
                for g in range(N_DC):
                    pq = ps_sp.tile([128, 512], F32, tag="ss", name=f"pq{qb}_{g}")
                    for dc in range(N_DC):
                        nc.tensor.matmul(pq[:], wts["M"][:, dc, g * 128 : (g + 1) * 128],
                                         xT[:, dc, qb * 512 : (qb + 1) * 512],
                                         start=(dc == 0), stop=(dc == N_DC - 1))
                    nc.scalar.activation(qT[:, g, :], pq[:], ActFn.Identity,
                                         bias=bqT[:, g : g + 1])
                return qT

            def emit_score(qb, st, qT):
                # scoreT[s in st, q] = sum_dc xT[:, dc, st]^T qT[:, dc, :]
                ss = ps_sp.tile([128, 512], F32, tag="ss", name=f"ss{qb}_{st}")
                for dc in range(N_DC):
                    nc.tensor.matmul(ss[:], xT[:, dc, st * 128 : (st + 1) * 128],
                                     qT[:, dc, :], start=(dc == 0), stop=(dc == N_DC - 1))
                return ss

            qT_cur = emit_qproj(0)
            # pre-load the ACT exp table AFTER the q~ copies in scalar engine
            # order: it costs 1.3us and is only needed by exp(st0)
            warm = const.tile([1, 1], F32, tag="warm")
            nc.scalar.activation(warm[:], ones_f32[0:1, 0:1], ActFn.Exp, scale=1.0)
            for qb in range(N_QB):
                qT = qT_cur
                l_sb = rlp.tile([128, 512], F32, tag="l_sb", name=f"lsb{qb}")
                # at[ct][c in chunk, q] accumulates (P x)^T for this query
                # block: at[c, q] = sum_k x[k, c] pT[k, q]
                at_ps = [ps_atp.tile([128, 512], F32, tag="at", name=f"at{qb}_{ct}")
                         for ct in range(4)]
                ss_q = [emit_score(qb, 0, qT), emit_score(qb, 1, qT)]
                for st in range(N_ST):
                    if qb == 0:
                        dma_hook(st)
                    if st + 2 < N_ST:
                        ss_q.append(emit_score(qb, st + 2, qT))
                    ss = ss_q.pop(0)
                    pT = pTp.tile([128, 512], MM_DT, tag="pT", name=f"pT{qb}_{st}")
                    nc.scalar.activation(pT[:], ss[:], ActFn.Exp, scale=SCALE)
                    for ct in range(4):
                        nc.tensor.matmul(at_ps[ct][:], xn[:, st, ct * 128 : (ct + 1) * 128],
                                         pT[:], start=(st == 0), stop=(st == N_ST - 1))
                    # row-sum accumulation on the DVE (off the PE)
                    if st == 0:
                        nc.vector.tensor_copy(l_sb[:], pT[:])
                    else:
                        nc.vector.tensor_add(l_sb[:], l_sb[:], pT[:])

                if qb + 1 < N_QB:
                    qT_cur = emit_qproj(qb + 1)

                # --- epilogue: 1/l arranged with queries on partitions.
                # One tiny bf16 matmul per 128-query block folds the partition
                # sum AND the transpose: lt[q, 0] = sum_p lbf[p, qt*128 + q] ---
                lbf = rlp.tile([128, 512], MM_DT, tag="lbf", name=f"lbf{qb}")
                nc.vector.tensor_copy(lbf[:], l_sb[:])
                lt_ps = ps_lp.tile([128, 4], F32, tag="l", name=f"lt{qb}")
                for qt in range(4):
                    nc.tensor.matmul(lt_ps[:, qt : qt + 1],
                                     lbf[:, qt * 128 : (qt + 1) * 128],
                                     ones_bf[:, 0:1])
                rlT = rlp.tile([128, 4], F32, tag="rlT", name=f"rlT{qb}")
                nc.vector.reciprocal(rlT[:], lt_ps[:])

                # (P x) to SBUF bf16, then apply N per 128-query block:
                # po[q, d] = sum_ct atb[:, ct, qt]^T N[:, ct, :], qt-outer so
                # po[0]'s scale + DMA overlap the remaining matmuls
                last = qb == N_QB - 1
                atb = atbp.tile([128, 4, 512], MM_DT, tag="atb", name=f"atb{qb}")
                for ct in range(4):
                    # for the final block split the copies across Scalar+Vector
                    # (no next-block exps to yield to), shortening the tail
                    if last and ct % 2:
                        nc.vector.tensor_copy(atb[:, ct, :], at_ps[ct][:])
                    else:
                        nc.scalar.activation(atb[:, ct, :], at_ps[ct][:], ActFn.Copy)
                for qt in range(4):
                    po = ps_atp.tile([128, 512], F32, tag="at", name=f"po{qb}_{qt}")
                    for ct in range(4):
                        nc.tensor.matmul(po[:], atb[:, ct, qt * 128 : (qt + 1) * 128],
                                         wts["N"][:, ct, :], start=(ct == 0), stop=(ct == 3))
                    ot = osbp.tile([128, D], MM_DT, tag="ot", name=f"ot{qb}_{qt}")
                    row = out[(qb * 4 + qt) * 128 : (qb * 4 + qt + 1) * 128, :]
                    if last:
                        nc.vector.tensor_scalar_mul(ot[:, 0:256], po[:, 0:256],
                                                    rlT[:, qt : qt + 1])
                        nc.scalar.activation(ot[:, 256:512], po[:, 256:512],
                                             ActFn.Copy, scale=rlT[:, qt : qt + 1])
                        eng1 = nc.scalar if qt < 2 else nc.sync
                        nc.sync.dma_start(row[:, 0:256], ot[:, 0:256])
                        eng1.dma_start(row[:, 256:512], ot[:, 256:512])
                    else:
                        nc.vector.tensor_scalar_mul(ot[:], po[:], rlT[:, qt : qt + 1])
                        eng = nc.sync if qt % 2 == 0 else nc.scalar
                        eng.dma_start(row, ot[:])

        const.release()
        persist.release()

    nc.compile()
    return nc


_NC_CACHE = None


def _get_nc():
    global _NC_CACHE
    if _NC_CACHE is None:
        _NC_CACHE = _build_program()
    return _NC_CACHE


def kernel(**inputs):
    f32 = np.float32
    x = np.asarray(inputs["x"], dtype=f32)
    xb16 = x.reshape(B, S, C).astype(ml_dtypes.bfloat16)
    # host-side transpose to xT[c, s] per batch
    xt = np.ascontiguousarray(xb16.transpose(0, 2, 1))

    def tile_x(xTb):
        # xT[c, s] -> [rg, dc, p, s] so each (rg, dc) chunk is contiguous
        return np.ascontiguousarray(
            xTb.reshape(N_DC, 128, N_RG, 512).transpose(2, 0, 1, 3))

    Wq = np.asarray(inputs["Wq"], dtype=f32)
    Wk = np.asarray(inputs["Wk"], dtype=f32)
    Wv = np.asarray(inputs["Wv"], dtype=f32)
    Wo = np.asarray(inputs["Wo"], dtype=f32)
    # host-side weight folds (exact algebra, f32):
    #   score = [x (Wq Wk^T) + bq Wk^T] x^T  (+ per-query const, cancels)
    #   out   = ((P x)/l) (Wv Wo) + (bv Wo + bo)
    M = np.ascontiguousarray(Wq @ Wk.T).astype(ml_dtypes.bfloat16)
    N = np.ascontiguousarray(Wv @ Wo).astype(ml_dtypes.bfloat16)
    bqf = np.ascontiguousarray(np.asarray(inputs["bq"], dtype=f32) @ Wk.T)
    bo_eff = np.asarray(inputs["bo"], dtype=f32) + np.asarray(inputs["bv"], dtype=f32) @ Wo

    in_maps = []
    for c in range(N_CORES):
        b, h = divmod(c, 2)
        # roll keys so this core's query half occupies rows 0..2047;
        # attention over keys is order-invariant so this is exact.
        if h:
            xnb = np.ascontiguousarray(
                np.concatenate([xb16[b][Q:], xb16[b][:Q]], axis=0))
            xTb = np.concatenate([xt[b][:, Q:], xt[b][:, :Q]], axis=1)
        else:
            xnb = np.ascontiguousarray(xb16[b])
            xTb = xt[b]
        in_maps.append({"x": tile_x(xTb), "xn": xnb, "bq": bqf, "M": M, "N": N})

    nc = _get_nc()
    try:
        res = run_bass_kernel_spmd(nc, in_maps, core_ids=list(range(N_CORES)))
    except Exception:
        # transient NRT/device hiccups recover on retry
        import time
        time.sleep(15)
        res = run_bass_kernel_spmd(nc, in_maps, core_ids=list(range(N_CORES)))

    out = np.empty((B, S, D), dtype=f32)
    for c in range(N_CORES):
        b, h = divmod(c, 2)
        out[b, h * Q : (h + 1) * Q] = np.asarray(res.results[c]["out"]).astype(f32)
    if np.any(bo_eff):
        out += bo_eff
    return out.reshape(B, 64, 64, D)
